# revision 6
# baseline (speedup 1.0000x reference)
"""Causal self-attention (B=2, S=2048, D=1024, H=16) on 8 TRN2 NeuronCores.

Sharding: batch (2) x head-group (4 heads each) -> 8 cores. Each core computes
Q/K/V projections for its 4 heads, causal flash-attention, and a partial
output projection (its 256 columns of the concatenated head outputs against
the matching rows of Wo^T). Host sums the 4 partials per batch and adds the
bias terms (bv @ Wo.T + bo), which are input-independent w.r.t. x.

On-chip layout per core (s = sequence, c = d_model contraction, d = head dims):
  xT   [1024, 2048]  x[b].T               (f32r)
  QT/KT [128, 2, 2048] heads packed 2-per-128-partitions (f16, scores matmul)
  V    [128, 16, 260] natural [s, d] with a ones column per head (65th col)
       so the PV matmul also produces the softmax denominator (f32r)
  scores^T tiles [k=128, q=512] in PSUM; exp via ACT (scale=1/8 fused);
  causal masking = multiplicative 0/1 mask on the 4 diagonal-straddle
  block shapes; 1/denom broadcast via gpsimd partition_broadcast.
"""

import numpy as np

N_CORES = 8
B, S, D = 2, 2048, 1024
H_PER_CORE = 4
DSL = 256  # d-slice per core (4 heads x 64)
NC_TILES = 8  # d_model / 128 contraction tiles
SCH = 512  # s-chunk
NSCH = S // SCH  # 4
NST = S // 128  # 16 s-tiles

_cache = {}


def _build(reps=1):
    import concourse.mybir as mybir
    import concourse.tile as tile
    from concourse import bacc

    f32 = mybir.dt.float32
    f32r = mybir.dt.float32r
    f16 = mybir.dt.float16
    EXP = mybir.ActivationFunctionType.Exp

    nc = bacc.Bacc("TRN2", target_bir_lowering=False, debug=False,
                   num_devices=N_CORES)

    xT = nc.dram_tensor("xT", [D, S], f32r, kind="ExternalInput").ap()
    wqT = nc.dram_tensor("wqT", [D, DSL], f32r, kind="ExternalInput").ap()
    wkT = nc.dram_tensor("wkT", [D, DSL], f32r, kind="ExternalInput").ap()
    wvT = nc.dram_tensor("wvT", [D, DSL], f32r, kind="ExternalInput").ap()
    woT = nc.dram_tensor("woT", [DSL, D], f32r, kind="ExternalInput").ap()
    bq2 = nc.dram_tensor("bq2", [128, 2], f32, kind="ExternalInput").ap()
    bk2 = nc.dram_tensor("bk2", [128, 2], f32, kind="ExternalInput").ap()
    y = nc.dram_tensor("y", [S, D], f32, kind="ExternalOutput").ap()

    with tile.TileContext(nc) as tc:
        import contextlib
        with contextlib.ExitStack() as ctx:
            singles = ctx.enter_context(tc.tile_pool(name="singles", bufs=1))
            work = ctx.enter_context(tc.tile_pool(name="work", bufs=1))

            # --- persistent SBUF tensors ---
            xt_sb = singles.tile([128, NC_TILES, S], f32r)
            wq_sb = singles.tile([128, NC_TILES, DSL], f32r)
            wk_sb = singles.tile([128, NC_TILES, DSL], f32r)
            wv_sb = singles.tile([128, NC_TILES, DSL], f32r)
            wo_sb = singles.tile([128, 2, D], f32r)
            bq_sb = singles.tile([128, 2], f32)
            bk_sb = singles.tile([128, 2], f32)
            qt_sb = work.tile([128, 2, S], f16)
            kt_sb = work.tile([128, 2, S], f16)
            v_sb = work.tile([128, NST, 260], f32r)
            att_sb = [work.tile([128, S], f32r, name=f"att{p}", tag=f"att{p}")
                      for p in range(2)]
            masks = [singles.tile([128, SCH], f32, name=f"mask{m}", tag=f"mask{m}")
                     for m in range(4)]

            # causal 0/1 masks for the 4 diagonal-straddle block offsets:
            # block row k (partition), col q; valid iff q - k - 128*m >= 0
            for m in range(4):
                nc.gpsimd.memset(masks[m], 1.0)
                nc.gpsimd.affine_select(
                    out=masks[m], in_=masks[m],
                    compare_op=mybir.AluOpType.is_ge, fill=0.0,
                    base=-128 * m, pattern=[[1, SCH]], channel_multiplier=-1)
            # ones columns of V (col 64 of each head slot); written once —
            # the per-rep V copies only touch cols 0..63 of each slot.
            # (f32r memset is unsupported; DVE copy from f32 rounds to f32r.)
            ones_f32 = singles.tile([128, 4], f32)
            nc.vector.memset(ones_f32, 1.0)
            v4 = v_sb.rearrange("p t (h e) -> p t h e", h=H_PER_CORE)
            for t in range(NST):
                nc.vector.tensor_copy(out=v4[:, t, :, 64:65],
                                      in_=ones_f32.rearrange("p (h e) -> p h e", h=4))

            body_pools = [None]

            def body(_iv=None):
                with contextlib.ExitStack() as bctx:
                    # --- load inputs ---
                    xt_in = xT.rearrange("(t p) s -> p t s", p=128)
                    for c in range(NC_TILES):
                        nc.sync.dma_start(out=xt_sb[:, c, :], in_=xt_in[:, c, :])
                    nc.sync.dma_start(out=wq_sb, in_=wqT.rearrange("(t p) d -> p t d", p=128))
                    nc.sync.dma_start(out=wk_sb, in_=wkT.rearrange("(t p) d -> p t d", p=128))
                    nc.sync.dma_start(out=wv_sb, in_=wvT.rearrange("(t p) d -> p t d", p=128))
                    nc.sync.dma_start(out=wo_sb, in_=woT.rearrange("(t p) d -> p t d", p=128))
                    nc.sync.dma_start(out=bq_sb, in_=bq2)
                    nc.sync.dma_start(out=bk_sb, in_=bk2)

                    # ================= Phase A: projections =================
                    pa = contextlib.ExitStack()
                    pp = pa.enter_context(tc.tile_pool(name="pp", bufs=4, space="PSUM"))
                    vp = pa.enter_context(tc.tile_pool(name="vp", bufs=4, space="PSUM"))
                    for sc in range(NSCH):
                        scs = slice(SCH * sc, SCH * (sc + 1))
                        qt_ps = [pp.tile([128, SCH], f32, name=f"qtp{h}", tag="qk")
                                 for h in range(2)]
                        kt_ps = [pp.tile([128, SCH], f32, name=f"ktp{h}", tag="qk")
                                 for h in range(2)]
                        v_ps = [vp.tile([128, DSL], f32, name=f"vp{t}", tag="v")
                                for t in range(4)]
                        for c in range(NC_TILES):
                            st = (c == 0)
                            sp = (c == NC_TILES - 1)
                            for half in range(2):
                                nc.tensor.matmul(
                                    qt_ps[half], lhsT=wq_sb[:, c, 128 * half:128 * (half + 1)],
                                    rhs=xt_sb[:, c, scs], start=st, stop=sp)
                                nc.tensor.matmul(
                                    kt_ps[half], lhsT=wk_sb[:, c, 128 * half:128 * (half + 1)],
                                    rhs=xt_sb[:, c, scs], start=st, stop=sp)
                            for t4 in range(4):
                                t = 4 * sc + t4
                                nc.tensor.matmul(
                                    v_ps[t4], lhsT=xt_sb[:, c, 128 * t:128 * (t + 1)],
                                    rhs=wv_sb[:, c, :], start=st, stop=sp)
                        for half in range(2):
                            nc.vector.tensor_scalar_add(
                                qt_sb[:, half, scs], qt_ps[half], bq_sb[:, half:half + 1])
                            nc.vector.tensor_scalar_add(
                                kt_sb[:, half, scs], kt_ps[half], bk_sb[:, half:half + 1])
                        for t4 in range(4):
                            t = 4 * sc + t4
                            nc.vector.tensor_copy(
                                out=v_sb.rearrange("p t (h e) -> p t h e", h=4)[:, t, :, 0:64],
                                in_=v_ps[t4].rearrange("p (h e) -> p h e", h=4))

                    pa.close()  # release projection PSUM banks

                    # ================= Phase B: attention + out-proj ========
                    sp_ = bctx.enter_context(tc.tile_pool(name="sp", bufs=3, space="PSUM"))
                    op_ = bctx.enter_context(tc.tile_pool(name="op", bufs=2, space="PSUM"))
                    yp = bctx.enter_context(tc.tile_pool(name="yp", bufs=2, space="PSUM"))
                    ep = bctx.enter_context(tc.tile_pool(name="ep", bufs=6))
                    bp = bctx.enter_context(tc.tile_pool(name="bp", bufs=4))
                    yo = bctx.enter_context(tc.tile_pool(name="yo", bufs=2))

                    for j in range(NSCH):
                        qs = slice(SCH * j, SCH * (j + 1))
                        T = 4 * (j + 1)
                        for pair in range(2):
                            o_ps = [op_.tile([65, SCH], f32, name=f"ops{h}", tag="o")
                                    for h in range(2)]
                            prev = None

                            def emit_pv(exps, t):
                                for h in range(2):
                                    hl = 2 * pair + h
                                    nc.tensor.matmul(
                                        o_ps[h], lhsT=v_sb[:, t, 65 * hl:65 * hl + 65],
                                        rhs=exps[h], start=(t == 0), stop=(t == T - 1))

                            for t in range(T):
                                s_ps = [sp_.tile([128, SCH], f32, name=f"sps{h}", tag="s")
                                        for h in range(2)]
                                for h in range(2):
                                    hp = slice(64 * h, 64 * (h + 1))
                                    nc.tensor.matmul(
                                        s_ps[h][:, :], lhsT=kt_sb[hp, pair, 128 * t:128 * (t + 1)],
                                        rhs=qt_sb[hp, pair, qs], start=True, stop=True)
                                exps = [ep.tile([128, SCH], f32r, name=f"exps{h}", tag="e")
                                        for h in range(2)]
                                for h in range(2):
                                    nc.scalar.activation(out=exps[h], in_=s_ps[h],
                                                         func=EXP, scale=0.125)
                                m = t - 4 * j
                                if m >= 0:
                                    for h in range(2):
                                        nc.vector.tensor_mul(exps[h], exps[h], masks[m])
                                if prev is not None:
                                    emit_pv(*prev)
                                prev = (exps, t)
                            emit_pv(*prev)

                            # normalize: att = O[0:64] * bcast(1/denom)
                            for h in range(2):
                                hl = 2 * pair + h
                                bc = bp.tile([128, SCH], f32, name=f"bc{h}", tag="bc")
                                nc.vector.reciprocal(out=bc[0:1, :], in_=o_ps[h][64:65, :])
                                nc.gpsimd.partition_broadcast(out_ap=bc[0:64, :], in_ap=bc[0:1, :])
                                nc.vector.tensor_mul(
                                    att_sb[pair][64 * h:64 * (h + 1), qs],
                                    o_ps[h][0:64, :], bc[0:64, :])

                        # out-projection for this q-chunk
                        for t4 in range(4):
                            t = 4 * j + t4
                            y_sb = yo.tile([128, D], f32, name="ysb", tag="ysb")
                            for e in range(2):
                                es = slice(512 * e, 512 * (e + 1))
                                y_ps = yp.tile([128, 512], f32, name="yps", tag="y")
                                for pair in range(2):
                                    nc.tensor.matmul(
                                        y_ps, lhsT=att_sb[pair][:, 128 * t:128 * (t + 1)],
                                        rhs=wo_sb[:, pair, es],
                                        start=(pair == 0), stop=(pair == 1))
                                nc.vector.tensor_copy(out=y_sb[:, es], in_=y_ps)
                            nc.sync.dma_start(out=y[128 * t:128 * (t + 1), :], in_=y_sb)

            if reps == 1:
                body()
            else:
                with tc.For_i(0, reps, 1):
                    body()

    nc.compile()
    return nc


def _get_nc(reps=1):
    if reps not in _cache:
        _cache[reps] = _build(reps)
    return _cache[reps]


def make_in_maps(x, Wq, bq, Wk, bk, Wv, bv, Wo, bo):
    """Shard full inputs into 8 per-core input dicts."""
    in_maps = []
    for core in range(N_CORES):
        b, g = core // 4, core % 4
        off = DSL * g
        sl = slice(off, off + DSL)
        in_maps.append({
            "xT": np.ascontiguousarray(x[b].T),
            "wqT": np.ascontiguousarray(Wq[sl, :].T),
            "wkT": np.ascontiguousarray(Wk[sl, :].T),
            "wvT": np.ascontiguousarray(Wv[sl, :].T),
            "woT": np.ascontiguousarray(Wo[:, sl].T),
            "bq2": np.ascontiguousarray(bq[sl].reshape(2, 128).T),
            "bk2": np.ascontiguousarray(bk[sl].reshape(2, 128).T),
        })
    return in_maps


def kernel(x, Wq, bq, Wk, bk, Wv, bv, Wo, bo):
    from concourse.bass_utils import run_bass_kernel_spmd

    x = np.asarray(x, dtype=np.float32)
    Wq, bq = np.asarray(Wq, np.float32), np.asarray(bq, np.float32)
    Wk, bk = np.asarray(Wk, np.float32), np.asarray(bk, np.float32)
    Wv, bv = np.asarray(Wv, np.float32), np.asarray(bv, np.float32)
    Wo, bo = np.asarray(Wo, np.float32), np.asarray(bo, np.float32)

    nc = _get_nc()
    in_maps = make_in_maps(x, Wq, bq, Wk, bk, Wv, bv, Wo, bo)
    res = run_bass_kernel_spmd(nc, in_maps, core_ids=list(range(N_CORES)))

    cvec = bv @ Wo.T + bo  # x-independent bias contribution
    out = np.zeros((B, S, D), dtype=np.float32)
    for core in range(N_CORES):
        out[core // 4] += res.results[core]["y"]
    out += cvec[None, None, :]
    return out


# revision 7
# speedup vs baseline: 1.9770x; 1.9770x over previous
"""Causal self-attention (B=2, S=2048, D=1024, H=16) on 8 TRN2 NeuronCores.

Sharding: batch (2) x head-group (4 heads each) -> 8 cores. Each core computes
Q/K/V projections for its 4 heads, causal flash-attention, and a partial
output projection (its 256 columns of the concatenated head outputs against
the matching rows of Wo^T). Host sums the 4 partials per batch and adds the
bias terms (bv @ Wo.T + bo), which are x-independent.

All large inputs are packed host-side into ONE [128, 24576] f32 tensor laid
out so every partition's data is contiguous in DRAM (128 big DMA descriptors
instead of thousands of small ones). Column map per partition p:
  [     0:16384)  xT   tiles: xt[p, c, s] = x[b].T[128c+p, s]   (8 x 2048)
  [16384:18432)  wqT  tiles: wq[p, c, d] = Wq.T[:, sl][128c+p, d] (8 x 256)
  [18432:20480)  wkT  same for Wk
  [20480:22528)  wvT  same for Wv
  [22528:24576)  woT  tiles: wo[p, t, e] = Wo.T[sl, :][128t+p, e] (2 x 1024)
Biases travel in a tiny [128, 4] side tensor (bq | bk halves).

Compute per core: scores^T = K^T q-major tiles via fp16 matmuls (2 heads
row-packed per 128 partitions), exp on ACT with the 1/8 scale fused, causal
masking as a multiplicative 0/1 mask on the 4 diagonal-straddle shapes,
PV matmul in f32r with a ones column appended to V so the softmax
denominator falls out of the same matmul, gpsimd partition_broadcast of
1/denom, and a final f32r out-projection against Wo^T rows.
"""

import numpy as np

N_CORES = 8
B, S, D = 2, 2048, 1024
H_PER_CORE = 4
DSL = 256
NC_TILES = 8
SCH = 512
NSCH = S // SCH
NST = S // 128

XT_O = 0
WQ_O = 16384
WK_O = WQ_O + 2048
WV_O = WK_O + 2048
WO_O = WV_O + 2048
IN_COLS = WO_O + 2048  # 24576

_cache = {}


def _build(reps=1):
    import contextlib
    import concourse.mybir as mybir
    import concourse.tile as tile
    from concourse import bacc

    f32 = mybir.dt.float32
    f32r = mybir.dt.float32r
    f16 = mybir.dt.float16
    EXP = mybir.ActivationFunctionType.Exp

    nc = bacc.Bacc("TRN2", target_bir_lowering=False, debug=False,
                   num_devices=N_CORES)

    big = nc.dram_tensor("big", [128, IN_COLS], f32r, kind="ExternalInput").ap()
    bqk = nc.dram_tensor("bqk", [128, 4], f32, kind="ExternalInput").ap()
    y = nc.dram_tensor("y", [S, D], f32, kind="ExternalOutput").ap()

    with tile.TileContext(nc) as tc:
        with contextlib.ExitStack() as ctx:
            singles = ctx.enter_context(tc.tile_pool(name="singles", bufs=1))
            work = ctx.enter_context(tc.tile_pool(name="work", bufs=1))

            big_sb = singles.tile([128, IN_COLS], f32r)
            xt_sb = big_sb[:, XT_O:WQ_O].rearrange("p (c s) -> p c s", c=NC_TILES)
            wq_sb = big_sb[:, WQ_O:WK_O].rearrange("p (c d) -> p c d", c=NC_TILES)
            wk_sb = big_sb[:, WK_O:WV_O].rearrange("p (c d) -> p c d", c=NC_TILES)
            wv_sb = big_sb[:, WV_O:WO_O].rearrange("p (c d) -> p c d", c=NC_TILES)
            wo_sb = big_sb[:, WO_O:IN_COLS].rearrange("p (t e) -> p t e", t=2)
            bqk_sb = singles.tile([128, 4], f32)

            qt_sb = work.tile([128, 2, S], f16)
            kt_sb = work.tile([128, 2, S], f16)
            v_sb = work.tile([128, NST, 260], f32r)
            att_sb = [work.tile([128, S], f32r, name=f"att{p}", tag=f"att{p}")
                      for p in range(2)]
            masks = [singles.tile([128, SCH], f32, name=f"mask{m}", tag=f"mask{m}")
                     for m in range(4)]

            # causal 0/1 masks: block row k (partition), col q;
            # valid iff q - k - 128*m >= 0
            for m in range(4):
                nc.gpsimd.memset(masks[m], 1.0)
                nc.gpsimd.affine_select(
                    out=masks[m], in_=masks[m],
                    compare_op=mybir.AluOpType.is_ge, fill=0.0,
                    base=-128 * m, pattern=[[1, SCH]], channel_multiplier=-1)
            # ones columns of V (col 64 of each head slot), written once:
            # per-rep V copies only touch cols 0..63 of each slot.
            ones_f32 = singles.tile([128, 4], f32)
            nc.vector.memset(ones_f32, 1.0)
            v4 = v_sb.rearrange("p t (h e) -> p t h e", h=H_PER_CORE)
            for t in range(NST):
                nc.vector.tensor_copy(out=v4[:, t, :, 64:65],
                                      in_=ones_f32.rearrange("p (h e) -> p h e", h=4))

            def body(_iv=None):
                with contextlib.ExitStack() as bctx:
                    # --- load inputs: x on the SP ring, weights on ACT ring ---
                    nc.sync.dma_start(out=big_sb[:, XT_O:WQ_O], in_=big[:, XT_O:WQ_O])
                    nc.scalar.dma_start(out=big_sb[:, WQ_O:IN_COLS], in_=big[:, WQ_O:IN_COLS])
                    nc.scalar.dma_start(out=bqk_sb, in_=bqk)

                    # ================= Phase A: projections =================
                    pa = contextlib.ExitStack()
                    pp = pa.enter_context(tc.tile_pool(name="pp", bufs=4, space="PSUM"))
                    vp = pa.enter_context(tc.tile_pool(name="vp", bufs=4, space="PSUM"))
                    for sc in range(NSCH):
                        scs = slice(SCH * sc, SCH * (sc + 1))
                        qt_ps = [pp.tile([128, SCH], f32, name=f"qtp{h}", tag="qk")
                                 for h in range(2)]
                        kt_ps = [pp.tile([128, SCH], f32, name=f"ktp{h}", tag="qk")
                                 for h in range(2)]
                        v_ps = [vp.tile([128, DSL], f32, name=f"vp{t}", tag="v")
                                for t in range(4)]
                        for c in range(NC_TILES):
                            st = (c == 0)
                            sp = (c == NC_TILES - 1)
                            for half in range(2):
                                nc.tensor.matmul(
                                    qt_ps[half], lhsT=wq_sb[:, c, 128 * half:128 * (half + 1)],
                                    rhs=xt_sb[:, c, scs], start=st, stop=sp)
                                nc.tensor.matmul(
                                    kt_ps[half], lhsT=wk_sb[:, c, 128 * half:128 * (half + 1)],
                                    rhs=xt_sb[:, c, scs], start=st, stop=sp)
                            for t4 in range(4):
                                t = 4 * sc + t4
                                nc.tensor.matmul(
                                    v_ps[t4], lhsT=xt_sb[:, c, 128 * t:128 * (t + 1)],
                                    rhs=wv_sb[:, c, :], start=st, stop=sp)
                        for half in range(2):
                            nc.vector.tensor_scalar_add(
                                qt_sb[:, half, scs], qt_ps[half], bqk_sb[:, half:half + 1])
                            nc.vector.tensor_scalar_add(
                                kt_sb[:, half, scs], kt_ps[half], bqk_sb[:, 2 + half:3 + half])
                        for t4 in range(4):
                            t = 4 * sc + t4
                            nc.vector.tensor_copy(
                                out=v4[:, t, :, 0:64],
                                in_=v_ps[t4].rearrange("p (h e) -> p h e", h=4))
                    pa.close()

                    # ================= Phase B: attention + out-proj ========
                    sp_ = bctx.enter_context(tc.tile_pool(name="sp", bufs=3, space="PSUM"))
                    op_ = bctx.enter_context(tc.tile_pool(name="op", bufs=2, space="PSUM"))
                    yp = bctx.enter_context(tc.tile_pool(name="yp", bufs=2, space="PSUM"))
                    ep = bctx.enter_context(tc.tile_pool(name="ep", bufs=6))
                    bp = bctx.enter_context(tc.tile_pool(name="bp", bufs=4))
                    yo = bctx.enter_context(tc.tile_pool(name="yo", bufs=2))

                    for j in range(NSCH):
                        qs = slice(SCH * j, SCH * (j + 1))
                        T = 4 * (j + 1)
                        for pair in range(2):
                            o_ps = [op_.tile([65, SCH], f32, name=f"ops{h}", tag="o")
                                    for h in range(2)]
                            prev = None

                            def emit_pv(exps, t):
                                for h in range(2):
                                    hl = 2 * pair + h
                                    nc.tensor.matmul(
                                        o_ps[h], lhsT=v_sb[:, t, 65 * hl:65 * hl + 65],
                                        rhs=exps[h], start=(t == 0), stop=(t == T - 1))

                            for t in range(T):
                                s_ps = [sp_.tile([128, SCH], f32, name=f"sps{h}", tag="s")
                                        for h in range(2)]
                                for h in range(2):
                                    hp = slice(64 * h, 64 * (h + 1))
                                    nc.tensor.matmul(
                                        s_ps[h][:, :], lhsT=kt_sb[hp, pair, 128 * t:128 * (t + 1)],
                                        rhs=qt_sb[hp, pair, qs], start=True, stop=True)
                                exps = [ep.tile([128, SCH], f32r, name=f"exps{h}", tag="e")
                                        for h in range(2)]
                                for h in range(2):
                                    nc.scalar.activation(out=exps[h], in_=s_ps[h],
                                                         func=EXP, scale=0.125)
                                m = t - 4 * j
                                if m >= 0:
                                    for h in range(2):
                                        nc.vector.tensor_mul(exps[h], exps[h], masks[m])
                                if prev is not None:
                                    emit_pv(*prev)
                                prev = (exps, t)
                            emit_pv(*prev)

                            # normalize: att = O[0:64] * bcast(1/denom)
                            for h in range(2):
                                bc = bp.tile([128, SCH], f32, name=f"bc{h}", tag="bc")
                                nc.vector.reciprocal(out=bc[0:1, :], in_=o_ps[h][64:65, :])
                                nc.gpsimd.partition_broadcast(out_ap=bc[0:64, :], in_ap=bc[0:1, :])
                                nc.vector.tensor_mul(
                                    att_sb[pair][64 * h:64 * (h + 1), qs],
                                    o_ps[h][0:64, :], bc[0:64, :])

                        # out-projection for this q-chunk (4 s-tiles), one DMA
                        y_sb = yo.tile([128, 4, D], f32, name="ysb", tag="ysb")
                        for t4 in range(4):
                            t = 4 * j + t4
                            for e in range(2):
                                es = slice(512 * e, 512 * (e + 1))
                                y_ps = yp.tile([128, 512], f32, name="yps", tag="y")
                                for pair in range(2):
                                    nc.tensor.matmul(
                                        y_ps, lhsT=att_sb[pair][:, 128 * t:128 * (t + 1)],
                                        rhs=wo_sb[:, pair, es],
                                        start=(pair == 0), stop=(pair == 1))
                                nc.vector.tensor_copy(out=y_sb[:, t4, es], in_=y_ps)
                        eng = nc.sync if j % 2 == 0 else nc.scalar
                        eng.dma_start(
                            out=y[SCH * j:SCH * (j + 1), :].rearrange("(t p) e -> p t e", p=128),
                            in_=y_sb)

            if reps == 1:
                body()
            else:
                with tc.For_i(0, reps, 1):
                    body()

    nc.compile()
    return nc


def _get_nc(reps=1):
    if reps not in _cache:
        _cache[reps] = _build(reps)
    return _cache[reps]


def _tiles(a, nt):
    # [nt*128, w] -> [128, nt*w] with [p, t*w:t*w+w] = a[128t+p, :]
    w = a.shape[1]
    return a.reshape(nt, 128, w).transpose(1, 0, 2).reshape(128, nt * w)


def make_in_maps(x, Wq, bq, Wk, bk, Wv, bv, Wo, bo):
    """Shard full inputs into 8 per-core input dicts."""
    in_maps = []
    for core in range(N_CORES):
        b, g = core // 4, core % 4
        sl = slice(DSL * g, DSL * (g + 1))
        big = np.concatenate([
            _tiles(np.ascontiguousarray(x[b].T), 8),
            _tiles(np.ascontiguousarray(Wq[sl, :].T), 8),
            _tiles(np.ascontiguousarray(Wk[sl, :].T), 8),
            _tiles(np.ascontiguousarray(Wv[sl, :].T), 8),
            _tiles(np.ascontiguousarray(Wo[:, sl].T), 2),
        ], axis=1)
        bqk = np.concatenate([bq[sl].reshape(2, 128).T, bk[sl].reshape(2, 128).T],
                             axis=1)
        in_maps.append({"big": big, "bqk": np.ascontiguousarray(bqk)})
    return in_maps


def kernel(x, Wq, bq, Wk, bk, Wv, bv, Wo, bo):
    from concourse.bass_utils import run_bass_kernel_spmd

    x = np.asarray(x, dtype=np.float32)
    Wq, bq = np.asarray(Wq, np.float32), np.asarray(bq, np.float32)
    Wk, bk = np.asarray(Wk, np.float32), np.asarray(bk, np.float32)
    Wv, bv = np.asarray(Wv, np.float32), np.asarray(bv, np.float32)
    Wo, bo = np.asarray(Wo, np.float32), np.asarray(bo, np.float32)

    nc = _get_nc()
    in_maps = make_in_maps(x, Wq, bq, Wk, bk, Wv, bv, Wo, bo)
    res = run_bass_kernel_spmd(nc, in_maps, core_ids=list(range(N_CORES)))

    cvec = bv @ Wo.T + bo  # x-independent bias contribution
    out = np.zeros((B, S, D), dtype=np.float32)
    for core in range(N_CORES):
        out[core // 4] += res.results[core]["y"]
    out += cvec[None, None, :]
    return out
